# revision 3
# baseline (speedup 1.0000x reference)
"""Trainium2 Bass kernel for nn_DifferentiableParallelBeamRadon.

Reference op: parallel-beam Radon transform of image [4,1,256,256] over 180
angles -> sinogram [4,1,180,256] (torch-style affine_grid/grid_sample bilinear
sampling with zeros padding, summed over rotated rows, scaled by 2/255).

Strategy (v2)
-------------
Geometry is input-independent.  Exact symmetries of the bilinear sampling grid
reduce the 180 angles to 46 coefficient tables (theta in [0,45] deg), each
serving up to 4 angles via index-transformed images:

    sino[t]      = R_t(img)                      V0
    sino[90-t]   = rev(R_t(img^T))               V1
    sino[90+t]   = R_t(fliplr(img^T))            V2
    sino[180-t]  = rev(R_t(flipud(img)))         V3

where R_t is the table-t operator sino[b,j] = sum_p sum_r C[r,p,j] *
V[b, p, X[p,j]+r] (row-binned tap windows, width R_t <= 4) and rev reverses
the detector axis.

The host quantizes the image to int8 (absmax scale, folded into the fp16
coefficients), gathers per-instance tap windows G (pure data layout), and
packs per core a single int8 blob: per slot one shared C table (fp16, stored
as raw bytes) + 4 instance G blocks (int8).  G is ~half the bytes of the
fp16 equivalent; C is shipped once per table instead of once per angle.

On device, per slot: C arrives via HWDGE (bitcast to fp16), G via GPSIMD
SWDGE DMAs that cast int8->fp16 in flight.  Per instance lane, an engine
(VectorE, optionally GPSIMD for load balance) forms P = C (*) G with C
broadcast along the batch axis, and TensorE reduces the 128 partitions with
ones-vector matmuls accumulating (h, r) planes in PSUM; ScalarE drains PSUM
to a staging row DMA'd out once per slot.
"""

import os

import numpy as np

IMAGE_SIZE = 256
NUM_ANGLES = 180
NUM_DET = 256
BATCH = 4
N_CORES = 8
R_MAX = 4
PAD = 4
WPAD = IMAGE_SIZE + 2 * PAD  # 264

N_TAB = 46                   # tables for theta = 0..45 deg
NSLOT = 6                    # ceil(46 / 8)
LANES = 4                    # instances per slot (one shared C table)

_DT_NP = np.float16


# ----------------------------------------------------------------------------
# geometry precompute (input independent, cached at import)
# ----------------------------------------------------------------------------

def _angle_tables(a_idx: int):
    """Return (axis, xidx int32 [256,256], C float64 [R_MAX,256,256])."""
    N = IMAGE_SIZE
    angles = np.linspace(0.0, 180.0, NUM_ANGLES + 1, dtype=np.float32)[:-1]
    ang = np.deg2rad(angles[a_idx], dtype=np.float32)
    cos = np.cos(ang, dtype=np.float32)
    sin = np.sin(ang, dtype=np.float32)

    j = np.arange(N, dtype=np.float32)
    xs = ((2.0 * j + 1.0) / np.float32(N) - 1.0).astype(np.float32)
    ys = xs.copy()

    gx = (cos * xs[None, :] + sin * ys[:, None]).astype(np.float32)
    gy = (-sin * xs[None, :] + cos * ys[:, None]).astype(np.float32)
    ix = (((gx + 1.0) * np.float32(N) - 1.0) * np.float32(0.5)).astype(np.float32)
    iy = (((gy + 1.0) * np.float32(N) - 1.0) * np.float32(0.5)).astype(np.float32)

    x0 = np.floor(ix)
    y0 = np.floor(iy)
    wx1 = (ix - x0).astype(np.float64)
    wy1 = (iy - y0).astype(np.float64)
    wx0 = 1.0 - wx1
    wy0 = 1.0 - wy1
    x0 = x0.astype(np.int64)
    y0 = y0.astype(np.int64)

    bin_by_row = abs(float(sin)) <= abs(float(cos))

    taps = [
        (y0, x0, wy0 * wx0),
        (y0, x0 + 1, wy0 * wx1),
        (y0 + 1, x0, wy1 * wx0),
        (y0 + 1, x0 + 1, wy1 * wx1),
    ]

    INF = 1 << 20
    qmin = np.full((N, N), INF, dtype=np.int64)
    qmax = np.full((N, N), -INF, dtype=np.int64)
    jj = np.broadcast_to(np.arange(N)[None, :], (N, N))
    binned = []
    for (rr, cc, w) in taps:
        valid = (rr >= 0) & (rr < N) & (cc >= 0) & (cc < N)
        bp, q = (rr, cc) if bin_by_row else (cc, rr)
        m = valid & (w > 0)
        binned.append((bp, q, w, m))
        np.minimum.at(qmin, (bp[m], jj[m]), q[m])
        np.maximum.at(qmax, (bp[m], jj[m]), q[m])

    width = np.where(qmin <= qmax, qmax - qmin + 1, 0)
    assert width.max() <= R_MAX, f"angle {a_idx}: window {width.max()}"
    qbase = np.where(qmin == INF, 0, qmin)

    C = np.zeros((R_MAX, N, N), dtype=np.float64)
    for (bp, q, w, m) in binned:
        r = q[m] - qbase[bp[m], jj[m]]
        np.add.at(C, (r, bp[m], jj[m]), w[m])

    C *= 2.0 / (IMAGE_SIZE - 1)
    return (0 if bin_by_row else 1), qbase.astype(np.int32), C


def _inst_map(a: int):
    """angle a -> (table, variant, detector-reversed)."""
    if a <= 45:
        return a, 0, False
    if a <= 89:
        return 90 - a, 1, True
    if a <= 135:
        return a - 90, 2, False
    return 180 - a, 3, True


_TABLES = None


def _get_tables():
    """Cached geometry + schedule.

    Per table t in [0,45]: R[t], FIDX[t] int32 [R_t,256,256] flat gather idx
    into the padded [256, 264] variant image, CDEV[t] float32 [128, 2*R_t*256]
    coefficient tile in device layout [pl, (h, r, j)].

    Schedule: TAB_GRID [NSLOT, N_CORES] -> table or -1, R_SLOT [NSLOT],
    INST[slot][core] -> list of up to 4 (angle, variant, rev).
    """
    global _TABLES
    if _TABLES is not None:
        return _TABLES

    r_tab = np.zeros(N_TAB, dtype=np.int64)
    fidx_all = []
    cdev_all = []
    for t in range(N_TAB):
        axis, xidx, C = _angle_tables(t)
        assert axis == 0, t
        nz = [r for r in range(R_MAX) if np.abs(C[r]).max() > 0]
        Rt = (max(nz) + 1) if nz else 1
        r_tab[t] = Rt
        rr = np.arange(Rt)[:, None, None]
        pp = np.arange(IMAGE_SIZE)[None, :, None]
        f = pp * WPAD + (xidx[None] + rr + PAD)
        assert f.min() >= 0 and f.max() < IMAGE_SIZE * WPAD
        fidx_all.append(f.astype(np.int32))
        # device layout [pl 128, (h 2, r Rt, j 256)]
        cl = C[:Rt].reshape(Rt, 2, 128, NUM_DET).transpose(2, 1, 0, 3)
        cdev_all.append(np.ascontiguousarray(cl.reshape(128, -1).astype(np.float32)))

    # instances per table
    tab_inst = [[] for _ in range(N_TAB)]
    for a in range(NUM_ANGLES):
        t, v, rev = _inst_map(a)
        tab_inst[t].append((a, v, rev))
    for t in range(N_TAB):
        assert len(tab_inst[t]) <= LANES

    # R-sorted (desc) round-robin table placement on the [NSLOT, N_CORES] grid
    order = np.argsort(-r_tab, kind="stable")
    tab_grid = np.full((NSLOT, N_CORES), -1, dtype=np.int64)
    for i, t in enumerate(order):
        tab_grid[i // N_CORES, i % N_CORES] = t
    r_slot = np.array(
        [max(max((r_tab[t] for t in row if t >= 0), default=1), 2) for row in tab_grid]
    )

    _TABLES = (r_tab, fidx_all, cdev_all, tab_inst, tab_grid, r_slot)
    return _TABLES


# ----------------------------------------------------------------------------
# bass program (built once, cached)
# ----------------------------------------------------------------------------

# per-partition byte layout per slot s (int8 blob):
#   C  raw fp16 bytes: 2*R_s*256*2   = 1024*R_s
#   G  int8:           LANES*2*R_s*256*BATCH = 8192*R_s
def _slot_sizes():
    _, _, _, _, _, r_slot = _get_tables()
    c_bytes = [int(r) * 1024 for r in r_slot]
    # lane stride in bytes: 2*R*256*BATCH
    lane_b = [int(r) * 2048 for r in r_slot]
    g_bytes = [LANES * lb for lb in lane_b]
    off = np.concatenate([[0], np.cumsum([c + g for c, g in zip(c_bytes, g_bytes)])])
    return c_bytes, g_bytes, lane_b, off


# engine for the elementwise multiply, per (slot, lane): 'v' = VectorE,
# 'g' = GPSIMD.  GPSIMD is ~4x slower (0.42 eff on 1.2GHz vs 2x mode on
# 0.96GHz) so it gets a small share for load balance.
def _mult_engines():
    env = os.environ.get("RADON_GPS_LANES", "")
    gps = set()
    if env:
        for tok in env.split(","):
            if tok:
                s, l = tok.split(":")
                gps.add((int(s), int(l)))
    else:
        gps = {(0, 3), (2, 3), (4, 3)}
    return gps


_PROG = {}


def _build_program(loop: int | None = None):
    if loop is None:
        loop = int(os.environ.get("RADON_LOOP", "0"))
    key = loop
    if key in _PROG:
        return _PROG[key]
    import concourse.bacc as bacc
    import concourse.mybir as mybir
    from concourse.tile import TileContext

    _, _, _, _, _, r_slot = _get_tables()
    c_bytes, g_bytes, lane_b, off = _slot_sizes()
    TOT = int(off[-1])

    REPEAT = int(os.environ.get("RADON_REPEAT", "1"))
    LOOP = loop
    gps_lanes = _mult_engines()

    dt = mybir.dt
    nbj = BATCH * NUM_DET

    nc = bacc.Bacc("TRN2", target_bir_lowering=False, debug=False,
                   num_devices=N_CORES)
    gc_dram = nc.dram_tensor("gc_in", [128, TOT], dt.int8,
                             kind="ExternalInput").ap()
    out_dram = nc.dram_tensor("sino_out", [1, NSLOT * LANES * nbj],
                              dt.float32, kind="ExternalOutput").ap()

    with TileContext(nc) as tc:
        with tc.tile_pool(name="const", bufs=1) as cpool, \
             tc.tile_pool(name="ctab", bufs=2) as ctab_pool, \
             tc.tile_pool(name="gdat", bufs=2) as g_pool, \
             tc.tile_pool(name="work", bufs=3) as p_pool, \
             tc.tile_pool(name="stage", bufs=2) as st_pool, \
             tc.tile_pool(name="psum", bufs=3, space="PSUM") as psum_pool:
            ones = cpool.tile([128, 1], dt.float16)
            nc.vector.memset(ones[:], 1.0)

            def _slot(s):
                Rs = int(r_slot[s])
                base = int(off[s])
                fc = 512 * Rs            # C fp16 elems per partition
                fl = 1024 * Rs * 2       # G fp16 elems per lane (b*h*r*j)
                # C: HWDGE int8 copy, bitcast to fp16
                ct = ctab_pool.tile([128, c_bytes[s]], dt.int8, tag="c")
                nc.sync.dma_start(
                    out=ct[:], in_=gc_dram[:, base:base + c_bytes[s]]
                )
                cw = ct[:].bitcast(dt.float16)  # [128, 512*Rs] = (h,r,j)
                gbase = base + c_bytes[s]
                st = st_pool.tile([1, LANES * nbj], dt.float32, tag="st")
                for half in range(2):
                    # G for 2 lanes: SWDGE cast int8 -> fp16
                    gt = g_pool.tile([128, 2 * fl], dt.float16, tag="g")
                    nc.gpsimd.dma_start(
                        out=gt[:],
                        in_=gc_dram[:, gbase + half * 2 * lane_b[s]:
                                    gbase + (half + 1) * 2 * lane_b[s]],
                    )
                    for sub in range(2):
                        lane = 2 * half + sub
                        g4 = gt[:, sub * fl:(sub + 1) * fl].rearrange(
                            "p (b h r j) -> p b h r j",
                            b=BATCH, h=2, r=Rs, j=NUM_DET,
                        )
                        cb = cw.rearrange(
                            "p (h r j) -> p h r j", h=2, r=Rs, j=NUM_DET
                        ).unsqueeze(1).to_broadcast([128, BATCH, 2, Rs, NUM_DET])
                        pt = p_pool.tile([128, fl], dt.float16, tag="p")
                        p4 = pt[:].rearrange(
                            "p (b h r j) -> p b h r j",
                            b=BATCH, h=2, r=Rs, j=NUM_DET,
                        )
                        eng = nc.gpsimd if (s, lane) in gps_lanes else nc.vector
                        eng.tensor_mul(out=p4, in0=cb, in1=g4)
                        ps = psum_pool.tile([1, nbj], dt.float32, space="PSUM")
                        ps3 = ps.rearrange("o (bp x) -> o bp x", bp=2, x=2 * NUM_DET)
                        p5 = pt[:].rearrange(
                            "p (bp c h r j) -> p bp c h r j",
                            bp=2, c=2, h=2, r=Rs, j=NUM_DET,
                        )
                        for bp in range(2):
                            n = 0
                            for h in range(2):
                                for r in range(Rs):
                                    nc.tensor.matmul(
                                        out=ps3[:, bp],
                                        lhsT=ones[:],
                                        rhs=p5[:, bp, :, h, r],
                                        start=(n == 0),
                                        stop=(n == 2 * Rs - 1),
                                    )
                                    n += 1
                        nc.scalar.copy(
                            out=st[:, lane * nbj:(lane + 1) * nbj], in_=ps[:]
                        )
                nc.sync.dma_start(
                    out=out_dram[:, s * LANES * nbj:(s + 1) * LANES * nbj],
                    in_=st[:],
                )

            def _body():
                for s in range(NSLOT):
                    _slot(s)

            if LOOP > 1:
                with tc.For_i(0, LOOP, 1):
                    _body()
            else:
                for _ in range(REPEAT):
                    _body()

    nc.finalize()
    _PROG[key] = (nc, TOT)
    return _PROG[key]


# ----------------------------------------------------------------------------
# entry point
# ----------------------------------------------------------------------------

def _host_pack(img: np.ndarray):
    """img [4,1,256,256] f32 -> per-core int8 blobs [128, TOT]."""
    r_tab, fidx_all, cdev_all, tab_inst, tab_grid, r_slot = _get_tables()
    c_bytes, g_bytes, lane_b, off = _slot_sizes()
    TOT = int(off[-1])

    im = np.asarray(img)[:, 0].astype(np.float32)
    absmax = float(np.abs(im).max())
    scale = absmax / 127.0 if absmax > 0 else 1.0
    q = np.clip(np.round(im / scale), -127, 127).astype(np.int8)

    # variant images, padded to [B, 256, WPAD]
    vars_ = [
        q,
        np.ascontiguousarray(q.transpose(0, 2, 1)),
        np.ascontiguousarray(q.transpose(0, 2, 1)[:, :, ::-1]),
        np.ascontiguousarray(q[:, ::-1, :]),
    ]
    flat = []
    for v in vars_:
        p = np.zeros((BATCH, IMAGE_SIZE, WPAD), dtype=np.int8)
        p[:, :, PAD:PAD + IMAGE_SIZE] = v
        flat.append(p.reshape(BATCH, -1))

    blobs = [np.zeros((128, TOT), dtype=np.int8) for _ in range(N_CORES)]
    for s in range(NSLOT):
        Rs = int(r_slot[s])
        base = int(off[s])
        for k in range(N_CORES):
            t = tab_grid[s, k]
            if t < 0:
                continue
            Rt = int(r_tab[t])
            # C block: fp16 bytes, scale folded in, padded r -> Rs
            cd = np.zeros((128, 2, Rs, NUM_DET), dtype=np.float16)
            cd[:, :, :Rt] = (
                cdev_all[t].reshape(128, 2, Rt, NUM_DET) * scale
            ).astype(np.float16)
            blobs[k][:, base:base + c_bytes[s]] = cd.reshape(128, -1).view(np.int8)
            # G blocks per lane
            for lane, (a, v, rev) in enumerate(tab_inst[t]):
                g = flat[v][:, fidx_all[t].ravel()]        # [4, Rt*256*256]
                g = g.reshape(BATCH, Rt, 2, 128, NUM_DET)
                gd = np.zeros((128, BATCH, 2, Rs, NUM_DET), dtype=np.int8)
                gd[:, :, :, :Rt] = g.transpose(3, 0, 2, 1, 4)
                lb = base + c_bytes[s] + lane * lane_b[s]
                blobs[k][:, lb:lb + lane_b[s]] = gd.reshape(128, -1)
    return blobs


def kernel(image: np.ndarray, _trace: bool = False):
    from concourse import bass_utils

    image = np.asarray(image)
    nc = _build_program(0)[0]
    r_tab, fidx_all, cdev_all, tab_inst, tab_grid, r_slot = _get_tables()
    blobs = _host_pack(image)

    in_maps = [{"gc_in": blobs[k]} for k in range(N_CORES)]
    res = bass_utils.run_bass_kernel_spmd(
        nc, in_maps, core_ids=list(range(N_CORES)), trace=_trace
    )

    sino = np.zeros((BATCH, 1, NUM_ANGLES, NUM_DET), dtype=np.float32)
    for k in range(N_CORES):
        o = res.results[k]["sino_out"].reshape(NSLOT, LANES, BATCH, NUM_DET)
        for s in range(NSLOT):
            t = tab_grid[s, k]
            if t < 0:
                continue
            for lane, (a, v, rev) in enumerate(tab_inst[t]):
                row = o[s, lane]
                sino[:, 0, a, :] = row[:, ::-1] if rev else row
    if _trace:
        return sino, res
    return sino


# revision 31
# speedup vs baseline: 1.3964x; 1.3964x over previous
"""Trainium2 Bass kernel for nn_DifferentiableParallelBeamRadon.

Reference op: parallel-beam Radon transform of image [4,1,256,256] over 180
angles -> sinogram [4,1,180,256] (torch-style affine_grid/grid_sample bilinear
sampling with zeros padding, summed over rotated rows, scaled by 2/255).

Strategy (v2)
-------------
Geometry is input-independent.  Exact symmetries of the bilinear sampling grid
reduce the 180 angles to 46 coefficient tables (theta in [0,45] deg), each
serving up to 4 angles via index-transformed images:

    sino[t]      = R_t(img)                      V0
    sino[90-t]   = rev(R_t(img^T))               V1
    sino[90+t]   = R_t(fliplr(img^T))            V2
    sino[180-t]  = rev(R_t(flipud(img)))         V3

where R_t is the table-t operator sino[b,j] = sum_p sum_r C[r,p,j] *
V[b, p, X[p,j]+r] (row-binned tap windows, width R_t <= 4) and rev reverses
the detector axis.

The host quantizes the image to int8 (absmax scale, folded into the fp16
coefficients), gathers per-instance tap windows G (pure data layout), and
packs per core a single int8 blob: all C tables first (fp16 stored as raw
bytes, preloaded once), then per slot 4 instance G blocks (int8).  G is half
the HBM bytes of fp16; C is shipped once per table instead of once per angle.

On device the work is split into 48 sub-units (slot x lane x batch-pair),
each routed through one of four engine paths chosen for load balance across
VectorE / ScalarE / GPSIMD / DMA (RADON_LANES):
  'a' int8 -> VectorE multiply (1x mode)
  'b' SWDGE cast-DMA int8->fp16 in flight -> VectorE multiply (2x)
  'c' int8 -> GPSIMD multiply
  'd' int8 -> ScalarE cast copy -> VectorE multiply (2x)
P = C (*) G (C broadcast along batch) is then reduced over the 128
partitions by TensorE ones-vector matmuls accumulating (h, r) planes in
PSUM; ScalarE drains PSUM to an fp16 staging row DMA'd out per slot.
Sub-units are issued as a produce/consume software pipeline (produce step
k+1 before consuming step k), and the timing For_i loop unrolls 4 bodies
so the pipeline fill/drain amortizes across iterations.
"""

import os

import numpy as np

IMAGE_SIZE = 256
NUM_ANGLES = 180
NUM_DET = 256
BATCH = 4
N_CORES = 8
R_MAX = 4
PAD = 4
WPAD = IMAGE_SIZE + 2 * PAD  # 264

N_TAB = 46                   # tables for theta = 0..45 deg
NSLOT = 6                    # ceil(46 / 8)
LANES = 4                    # instances per slot (one shared C table)

_DT_NP = np.float16


# ----------------------------------------------------------------------------
# geometry precompute (input independent, cached at import)
# ----------------------------------------------------------------------------

def _angle_tables(a_idx: int):
    """Return (axis, xidx int32 [256,256], C float64 [R_MAX,256,256])."""
    N = IMAGE_SIZE
    angles = np.linspace(0.0, 180.0, NUM_ANGLES + 1, dtype=np.float32)[:-1]
    ang = np.deg2rad(angles[a_idx], dtype=np.float32)
    cos = np.cos(ang, dtype=np.float32)
    sin = np.sin(ang, dtype=np.float32)

    j = np.arange(N, dtype=np.float32)
    xs = ((2.0 * j + 1.0) / np.float32(N) - 1.0).astype(np.float32)
    ys = xs.copy()

    gx = (cos * xs[None, :] + sin * ys[:, None]).astype(np.float32)
    gy = (-sin * xs[None, :] + cos * ys[:, None]).astype(np.float32)
    ix = (((gx + 1.0) * np.float32(N) - 1.0) * np.float32(0.5)).astype(np.float32)
    iy = (((gy + 1.0) * np.float32(N) - 1.0) * np.float32(0.5)).astype(np.float32)

    x0 = np.floor(ix)
    y0 = np.floor(iy)
    wx1 = (ix - x0).astype(np.float64)
    wy1 = (iy - y0).astype(np.float64)
    wx0 = 1.0 - wx1
    wy0 = 1.0 - wy1
    x0 = x0.astype(np.int64)
    y0 = y0.astype(np.int64)

    bin_by_row = abs(float(sin)) <= abs(float(cos))

    taps = [
        (y0, x0, wy0 * wx0),
        (y0, x0 + 1, wy0 * wx1),
        (y0 + 1, x0, wy1 * wx0),
        (y0 + 1, x0 + 1, wy1 * wx1),
    ]

    INF = 1 << 20
    qmin = np.full((N, N), INF, dtype=np.int64)
    qmax = np.full((N, N), -INF, dtype=np.int64)
    jj = np.broadcast_to(np.arange(N)[None, :], (N, N))
    binned = []
    for (rr, cc, w) in taps:
        valid = (rr >= 0) & (rr < N) & (cc >= 0) & (cc < N)
        bp, q = (rr, cc) if bin_by_row else (cc, rr)
        m = valid & (w > 0)
        binned.append((bp, q, w, m))
        np.minimum.at(qmin, (bp[m], jj[m]), q[m])
        np.maximum.at(qmax, (bp[m], jj[m]), q[m])

    width = np.where(qmin <= qmax, qmax - qmin + 1, 0)
    assert width.max() <= R_MAX, f"angle {a_idx}: window {width.max()}"
    qbase = np.where(qmin == INF, 0, qmin)

    C = np.zeros((R_MAX, N, N), dtype=np.float64)
    for (bp, q, w, m) in binned:
        r = q[m] - qbase[bp[m], jj[m]]
        np.add.at(C, (r, bp[m], jj[m]), w[m])

    C *= 2.0 / (IMAGE_SIZE - 1)
    return (0 if bin_by_row else 1), qbase.astype(np.int32), C


def _inst_map(a: int):
    """angle a -> (table, variant, detector-reversed)."""
    if a <= 45:
        return a, 0, False
    if a <= 89:
        return 90 - a, 1, True
    if a <= 135:
        return a - 90, 2, False
    return 180 - a, 3, True


_TABLES = None


def _get_tables():
    """Cached geometry + schedule.

    Per table t in [0,45]: R[t], FIDX[t] int32 [R_t,256,256] flat gather idx
    into the padded [256, 264] variant image, CDEV[t] float32 [128, 2*R_t*256]
    coefficient tile in device layout [pl, (h, r, j)].

    Schedule: TAB_GRID [NSLOT, N_CORES] -> table or -1, R_SLOT [NSLOT],
    INST[slot][core] -> list of up to 4 (angle, variant, rev).
    """
    global _TABLES
    if _TABLES is not None:
        return _TABLES

    r_tab = np.zeros(N_TAB, dtype=np.int64)
    fidx_all = []
    cdev_all = []
    for t in range(N_TAB):
        axis, xidx, C = _angle_tables(t)
        assert axis == 0, t
        nz = [r for r in range(R_MAX) if np.abs(C[r]).max() > 0]
        Rt = (max(nz) + 1) if nz else 1
        r_tab[t] = Rt
        rr = np.arange(Rt)[:, None, None]
        pp = np.arange(IMAGE_SIZE)[None, :, None]
        f = pp * WPAD + (xidx[None] + rr + PAD)
        assert f.min() >= 0 and f.max() < IMAGE_SIZE * WPAD
        fidx_all.append(f.astype(np.int32))
        # device layout [pl 128, (h 2, r Rt, j 256)]
        cl = C[:Rt].reshape(Rt, 2, 128, NUM_DET).transpose(2, 1, 0, 3)
        cdev_all.append(np.ascontiguousarray(cl.reshape(128, -1).astype(np.float32)))

    # instances per table
    tab_inst = [[] for _ in range(N_TAB)]
    for a in range(NUM_ANGLES):
        t, v, rev = _inst_map(a)
        tab_inst[t].append((a, v, rev))
    for t in range(N_TAB):
        assert len(tab_inst[t]) <= LANES

    # R-sorted (desc) round-robin table placement on the [NSLOT, N_CORES] grid
    order = np.argsort(-r_tab, kind="stable")
    tab_grid = np.full((NSLOT, N_CORES), -1, dtype=np.int64)
    for i, t in enumerate(order):
        tab_grid[i // N_CORES, i % N_CORES] = t
    r_slot = np.array(
        [max(max((r_tab[t] for t in row if t >= 0), default=1), 2) for row in tab_grid]
    )

    _TABLES = (r_tab, fidx_all, cdev_all, tab_inst, tab_grid, r_slot)
    return _TABLES


# ----------------------------------------------------------------------------
# bass program (built once, cached)
# ----------------------------------------------------------------------------

# per-partition byte layout of the int8 blob:
#   [ all C blocks, slot-major:  2*R_s*256 fp16 -> 1024*R_s bytes each ]
#   [ per slot: LANES G blocks:  2*R_s*256*BATCH int8 -> 2048*R_s each ]
def _slot_sizes():
    _, _, _, _, _, r_slot = _get_tables()
    c_bytes = [int(r) * 1024 for r in r_slot]
    c_off = np.concatenate([[0], np.cumsum(c_bytes)])
    # lane stride in bytes: 2*R*256*BATCH
    lane_b = [int(r) * 2048 for r in r_slot]
    g_off = np.concatenate(
        [[int(c_off[-1])], int(c_off[-1]) + np.cumsum([LANES * lb for lb in lane_b])]
    )
    return c_bytes, c_off, lane_b, g_off


# Per-lane compute route (same for every core, SPMD):
#   'a' int8 G in SBUF, VectorE mult in 1x mode        (DVE 170.9 us/unit)
#   'b' SWDGE cast-DMA to fp16, VectorE mult 2x        (DMA-heavy)
#   'c' int8 G in SBUF, GPSIMD mult                    (GPS 325.6 us/unit)
#   'd' int8 G in SBUF, ScalarE cast copy, DVE mult 2x (Scalar+DVE)
def _lane_types():
    """8 chars per slot: one type per (lane, batch-pair) sub-unit."""
    env = os.environ.get("RADON_LANES", "")
    if env:
        rows = env.split(";")
        out = []
        for r in rows:
            out.append(list(r) if len(r) == 2 * LANES else
                       [c for c in r for _ in range(2)])
        return out
    return [list("bbbdddcb")] * NSLOT


_PROG = {}


def _build_program(loop: int | None = None):
    if loop is None:
        loop = int(os.environ.get("RADON_LOOP", "0"))
    key = loop
    if key in _PROG:
        return _PROG[key]
    import concourse.bacc as bacc
    import concourse.mybir as mybir
    from concourse.tile import TileContext

    _, _, _, _, _, r_slot = _get_tables()
    c_bytes, c_off, lane_b, g_off = _slot_sizes()
    TOT = int(g_off[-1])
    CTOT = int(c_off[-1])

    REPEAT = int(os.environ.get("RADON_REPEAT", "1"))
    LOOP = loop
    lane_types = _lane_types()

    dt = mybir.dt
    nbj = BATCH * NUM_DET

    nc = bacc.Bacc("TRN2", target_bir_lowering=False, debug=False,
                   num_devices=N_CORES)
    gc_dram = nc.dram_tensor("gc_in", [128, TOT], dt.int8,
                             kind="ExternalInput").ap()
    out_dram = nc.dram_tensor("sino_out", [1, NSLOT * LANES * nbj],
                              dt.float16, kind="ExternalOutput").ap()

    with TileContext(nc) as tc:
        with tc.tile_pool(name="const", bufs=1) as cpool, \
             tc.tile_pool(name="gi8", bufs=8) as gi_pool, \
             tc.tile_pool(name="gf16", bufs=5) as gfb_pool, \
             tc.tile_pool(name="gfd16", bufs=4) as gf_pool, \
             tc.tile_pool(name="work", bufs=8) as p_pool, \
             tc.tile_pool(name="stage", bufs=2) as st_pool, \
             tc.tile_pool(name="psum", bufs=8, space="PSUM") as psum_pool:
            ones = cpool.tile([128, 1], dt.float16)
            nc.vector.memset(ones[:], 1.0)
            # all C tables up front in one transfer, resident for the run
            c_all = cpool.tile([128, CTOT], dt.int8)
            nc.sync.dma_start(out=c_all[:], in_=gc_dram[:, :CTOT])

            def _produce(s, u, typ):
                """DMA (+cast) + elementwise multiply of sub-unit u;
                returns the P tile [128, 2*2*Rs*256] fp16."""
                Rs = int(r_slot[s])
                fs = 1024 * Rs
                cw = c_all[:, int(c_off[s]):int(c_off[s]) + c_bytes[s]] \
                    .bitcast(dt.float16)
                cb = cw.rearrange(
                    "p (h r j) -> p h r j", h=2, r=Rs, j=NUM_DET
                ).unsqueeze(1).to_broadcast([128, 2, 2, Rs, NUM_DET])
                g0 = int(g_off[s]) + u * fs
                gsl = gc_dram[:, g0:g0 + fs]
                if typ == "b":
                    gt = gfb_pool.tile([128, fs], dt.float16, tag="gf")
                    nc.gpsimd.dma_start(out=gt[:], in_=gsl)
                    gv = gt[:]
                    eng = nc.vector
                elif typ == "d":
                    gi = gi_pool.tile([128, fs], dt.int8, tag="gi")
                    dma_eng = nc.scalar if u % 2 else nc.sync
                    dma_eng.dma_start(out=gi[:], in_=gsl)
                    gt = gf_pool.tile([128, fs], dt.float16, tag="gd")
                    nc.scalar.copy(out=gt[:], in_=gi[:])
                    gv = gt[:]
                    eng = nc.vector
                else:  # 'a' (DVE int8) or 'c' (GPSIMD int8)
                    gi = gi_pool.tile([128, fs], dt.int8, tag="gi")
                    dma_eng = nc.scalar if u % 2 else nc.sync
                    dma_eng.dma_start(out=gi[:], in_=gsl)
                    gv = gi[:]
                    eng = nc.vector if typ == "a" else nc.gpsimd
                g4 = gv.rearrange(
                    "p (b h r j) -> p b h r j", b=2, h=2, r=Rs, j=NUM_DET
                )
                pt = p_pool.tile([128, fs], dt.float16, tag="p")
                p4 = pt[:].rearrange(
                    "p (b h r j) -> p b h r j", b=2, h=2, r=Rs, j=NUM_DET
                )
                eng.tensor_mul(out=p4, in0=cb, in1=g4)
                return pt

            def _consume(s, lane, bp, pt, st):
                Rs = int(r_slot[s])
                ps = psum_pool.tile([1, 2 * NUM_DET], dt.float32, space="PSUM")
                p4 = pt[:].rearrange(
                    "p (b h r j) -> p b h r j", b=2, h=2, r=Rs, j=NUM_DET
                )
                n = 0
                for h in range(2):
                    for r in range(Rs):
                        nc.tensor.matmul(
                            out=ps[:],
                            lhsT=ones[:],
                            rhs=p4[:, :, h, r],
                            start=(n == 0),
                            stop=(n == 2 * Rs - 1),
                        )
                        n += 1
                o = (lane * 2 + bp) * 2 * NUM_DET
                nc.scalar.copy(out=st[:, o:o + 2 * NUM_DET], in_=ps[:])

            # sub-unit list per slot: GPSIMD first in produce order (long
            # pole), last in consume order
            def _units(s):
                u = [(lane, bp, lane_types[s][2 * lane + bp])
                     for lane in range(LANES) for bp in range(2)]
                return sorted(u, key=lambda x: 0 if x[2] == "c" else 1)

            def _body_multi(U):
                # software pipeline across U unrolled repetitions: produce
                # step k+1 before consuming step k so no engine head-of-line
                # blocks another and fill/drain amortizes over U iterations
                seq = [(it, s) for it in range(U) for s in range(NSLOT)]
                prod = {}

                def _prod_step(k):
                    it, s = seq[k]
                    for lane, bp, typ in _units(s):
                        prod[(s, lane, bp)] = _produce(s, 2 * lane + bp, typ)

                _prod_step(0)
                for k, (it, s) in enumerate(seq):
                    if k + 1 < len(seq):
                        _prod_step(k + 1)
                    st = st_pool.tile([1, LANES * nbj], dt.float16, tag="st")
                    for lane, bp, typ in sorted(_units(s), key=lambda x: (
                            0 if x[2] != "c" else 1, x[0], x[1])):
                        _consume(s, lane, bp, prod.pop((s, lane, bp)), st)
                    (nc.scalar if s % 2 else nc.sync).dma_start(
                        out=out_dram[:, s * LANES * nbj:(s + 1) * LANES * nbj],
                        in_=st[:],
                    )

            if LOOP > 1:
                UNROLL = int(os.environ.get("RADON_UNROLL", "8"))
                while LOOP % UNROLL:
                    UNROLL -= 1
                with tc.For_i(0, LOOP // UNROLL, 1):
                    _body_multi(UNROLL)
            else:
                _body_multi(max(1, REPEAT))

    nc.finalize()
    _PROG[key] = (nc, TOT)
    return _PROG[key]


# ----------------------------------------------------------------------------
# entry point
# ----------------------------------------------------------------------------

def _host_pack(img: np.ndarray):
    """img [4,1,256,256] f32 -> per-core int8 blobs [128, TOT]."""
    r_tab, fidx_all, cdev_all, tab_inst, tab_grid, r_slot = _get_tables()
    c_bytes, c_off, lane_b, g_off = _slot_sizes()
    TOT = int(g_off[-1])

    im = np.asarray(img)[:, 0].astype(np.float32)
    absmax = float(np.abs(im).max())
    scale = absmax / 127.0 if absmax > 0 else 1.0
    q = np.clip(np.round(im / scale), -127, 127).astype(np.int8)

    # variant images, padded to [B, 256, WPAD]
    vars_ = [
        q,
        np.ascontiguousarray(q.transpose(0, 2, 1)),
        np.ascontiguousarray(q.transpose(0, 2, 1)[:, :, ::-1]),
        np.ascontiguousarray(q[:, ::-1, :]),
    ]
    flat = []
    for v in vars_:
        p = np.zeros((BATCH, IMAGE_SIZE, WPAD), dtype=np.int8)
        p[:, :, PAD:PAD + IMAGE_SIZE] = v
        flat.append(p.reshape(BATCH, -1))

    blobs = [np.zeros((128, TOT), dtype=np.int8) for _ in range(N_CORES)]
    for s in range(NSLOT):
        Rs = int(r_slot[s])
        for k in range(N_CORES):
            t = tab_grid[s, k]
            if t < 0:
                continue
            Rt = int(r_tab[t])
            # C block: fp16 bytes, scale folded in, padded r -> Rs
            cd = np.zeros((128, 2, Rs, NUM_DET), dtype=np.float16)
            cd[:, :, :Rt] = (
                cdev_all[t].reshape(128, 2, Rt, NUM_DET) * scale
            ).astype(np.float16)
            cb = int(c_off[s])
            blobs[k][:, cb:cb + c_bytes[s]] = cd.reshape(128, -1).view(np.int8)
            # G blocks per lane
            for lane, (a, v, rev) in enumerate(tab_inst[t]):
                g = flat[v][:, fidx_all[t].ravel()]        # [4, Rt*256*256]
                g = g.reshape(BATCH, Rt, 2, 128, NUM_DET)
                gd = np.zeros((128, BATCH, 2, Rs, NUM_DET), dtype=np.int8)
                gd[:, :, :, :Rt] = g.transpose(3, 0, 2, 1, 4)
                lb = int(g_off[s]) + lane * lane_b[s]
                blobs[k][:, lb:lb + lane_b[s]] = gd.reshape(128, -1)
    return blobs


def kernel(image: np.ndarray, _trace: bool = False):
    from concourse import bass_utils

    image = np.asarray(image)
    nc = _build_program(0)[0]
    r_tab, fidx_all, cdev_all, tab_inst, tab_grid, r_slot = _get_tables()
    blobs = _host_pack(image)

    in_maps = [{"gc_in": blobs[k]} for k in range(N_CORES)]
    res = bass_utils.run_bass_kernel_spmd(
        nc, in_maps, core_ids=list(range(N_CORES)), trace=_trace
    )

    sino = np.zeros((BATCH, 1, NUM_ANGLES, NUM_DET), dtype=np.float32)
    for k in range(N_CORES):
        o = res.results[k]["sino_out"].astype(np.float32).reshape(
            NSLOT, LANES, BATCH, NUM_DET)
        for s in range(NSLOT):
            t = tab_grid[s, k]
            if t < 0:
                continue
            for lane, (a, v, rev) in enumerate(tab_inst[t]):
                row = o[s, lane]
                sino[:, 0, a, :] = row[:, ::-1] if rev else row
    if _trace:
        return sino, res
    return sino
